# revision 15
# baseline (speedup 1.0000x reference)
"""Trainium2 Bass kernel for DirCFConv-style GNN message passing.

Computes, for inputs s:(B,N,H) f32, ef_mask:(B,N,N,H) f32, W:(H,H), b:(H,):
    m   = SiLU(LayerNorm(s @ W.T + b))          # (B,N,H)
    out[b,i,h] = sum_j ef_mask[b,i,j,h] * m[b,j,h]

Sharding: 8 cores, core c handles batch b = c // 2 and query-node half
i in [ (c%2)*256, (c%2)*256+256 ).  Each core streams its 64 MiB mask
shard from HBM (memory-bound).

Mask layout: partition p holds the j-quad j in [4p, 4p+4), so each
per-partition DMA chunk is (a h) = 4*128 f32 = 2 KiB *contiguous* in
HBM (vs 512 B for the j-partition layout) - 4x fewer descriptors and
near-line-rate HBM reads.  m is reshuffled once into the matching
[p, (a h)] layout via a scratch-DRAM roundtrip, then replicated ISUB
times so one tensor_mul covers a whole mask tile.  The j-reduction is
4 PE matmuls per i (one per a-slot) accumulating into one PSUM column.
"""

import numpy as np

import concourse.bass as bass
import concourse.bacc as bacc
import concourse.tile as tile
from concourse import mybir
from concourse.bass_utils import run_bass_kernel_spmd
from concourse.masks import make_identity

B, N, H = 4, 512, 128
P = 128
NJT = N // P          # 4 j-tiles of 128
AQ = 4                # j's per partition (quad)
ISUB = 8              # i's per mask tile -> 2 MiB DMAs
IH = N // 2           # 256 i's per core
N_CORES = 8
LN_EPS = 1e-5
F32 = mybir.dt.float32
BF16 = mybir.dt.bfloat16


def build_nc(ih=IH, prod_dtype=F32, repeat=1, do_mul=True, do_mm=True,
             debug_mrep=False):
    nc = bacc.Bacc()
    s_d = nc.declare_dram_parameter("s", [N, H], F32, isOutput=False)
    w_d = nc.declare_dram_parameter("w", [H, H], F32, isOutput=False)
    b_d = nc.declare_dram_parameter("b", [H], F32, isOutput=False)
    mask_d = nc.declare_dram_parameter("mask", [ih, N, H], F32, isOutput=False)
    out_d = nc.declare_dram_parameter("out", [ih, H], F32, isOutput=True)
    dbg_d = None
    if debug_mrep and (debug_mrep & 1):
        dbg_d = nc.declare_dram_parameter(
            "dbg", [P, ISUB, AQ * H], F32, isOutput=True
        )

    nit = ih // ISUB
    with tile.TileContext(nc) as tc:
        with (
            tc.tile_pool(name="consts", bufs=1) as consts,
            tc.tile_pool(name="small", bufs=4) as small,
            tc.tile_pool(name="loads", bufs=5) as loads,
            tc.tile_pool(name="prod", bufs=4) as prod,
            tc.tile_pool(name="outs", bufs=3) as outs,
        ):
            stage1_psum = tc.tile_pool(name="spsum", bufs=1, space="PSUM")
            spsum = stage1_psum.__enter__()
            # ---------------- constants ----------------
            # All constants are produced on gpsimd BEFORE make_identity so the
            # single carrier wait (Pool sem) covers every one of them.
            ones_col = consts.tile([P, 1], BF16)
            nc.gpsimd.memset(ones_col, 1.0)
            ones_row = consts.tile([1, P], F32)
            nc.gpsimd.memset(ones_row, 1.0)
            # eps on DVE: its consumer (ACT Sqrt) already waits on DVE for mv,
            # and one DVE sem wait covers both (Activation also allows only 1).
            eps_t = consts.tile([P, 1], F32)
            nc.vector.memset(eps_t, LN_EPS)
            ident = consts.tile([P, P], F32)
            make_identity(nc, ident)

            # Small parameter loads ride the ACT HWDGE ring so the SP ring
            # carries only the big mask streams (first mask DMA starts at t=0).
            w_sb = consts.tile([H, H], F32)
            nc.scalar.dma_start(out=w_sb, in_=w_d[:, :])
            bias_sb = consts.tile([1, H], F32)
            b_ap = b_d[:]
            bias_src = bass.AP(
                tensor=b_ap.tensor, offset=b_ap.offset, ap=[[0, 1]] + list(b_ap.ap)
            )
            nc.scalar.dma_start(out=bias_sb, in_=bias_src)

            # Wait-carrier: walrus allows only ONE sync wait per Matmult, so
            # absorb the gpsimd(identity) dependency into a throwaway PE op;
            # later matmuls then only carry their own single DMA/engine wait.
            carrier_ps = spsum.tile([P, P], F32)
            nc.tensor.transpose(carrier_ps, ident, ident)

            # W^T via PE-transpose: (o,h) -> (h,o)
            wT_ps = spsum.tile([H, H], F32)
            nc.tensor.transpose(wT_ps, w_sb, ident)
            wT_sb = consts.tile([H, H], F32)
            nc.scalar.copy(wT_sb, wT_ps)

            # ------------- m = SiLU(LN(s @ W.T + b)) -------------
            # All four s^T blocks share one PSUM bank (one zero-region group);
            # likewise the four h = s@W.T+b blocks.  No PSUM slot rotation ->
            # no extra release waits on any Matmult.
            sT_all = spsum.tile([P, NJT * P], F32)
            h_all = spsum.tile([P, NJT * H], F32)
            s_sbs = []
            for jt in range(NJT):
                s_sb = small.tile([P, H], F32, tag=f"s_sb{jt}")
                nc.scalar.dma_start(out=s_sb, in_=s_d[jt * P:(jt + 1) * P, :])
                s_sbs.append(s_sb)
                nc.tensor.matmul(
                    sT_all[:, jt * P:(jt + 1) * P],
                    lhsT=s_sb,
                    rhs=ident,
                    is_transpose=True,
                    start=(jt == 0),
                    stop=(jt == NJT - 1),
                )
            sT_sb = consts.tile([P, NJT * P], F32)
            nc.scalar.copy(sT_sb, sT_all)
            # h_quad[p, a*H+h] = h[4p+a, h]: the a-th matmul uses the
            # free-dim-strided stationary sT_sb[:, a::4] (columns j=4p+a),
            # so m lands directly in the j-quad layout the mask tiles use.
            for a in range(AQ):
                nc.tensor.matmul(
                    h_all[:, a * H:(a + 1) * H],
                    lhsT=sT_sb[:, a::AQ],
                    rhs=wT_sb,
                    start=(a == 0),
                    stop=False,
                )
                nc.tensor.matmul(
                    h_all[:, a * H:(a + 1) * H],
                    lhsT=ones_row,
                    rhs=bias_sb,
                    start=False,
                    stop=(a == AQ - 1),
                )

            # LN + SiLU per a-slot, writing m_rep[p, 0, (a h)] = m[4p+a, h]
            # (bf16: the product path runs in bf16 so PE stationary loads are
            # 1 cyc/row instead of fp32r's 4, and DVE gets the 16-bit rate)
            m_rep = consts.tile([P, ISUB, AQ * H], BF16)
            for a in range(AQ):
                h_ps = h_all[:, a * H:(a + 1) * H]
                stats = small.tile([P, 6], F32)
                nc.vector.bn_stats(stats, h_ps)
                mv = small.tile([P, 2], F32)
                nc.vector.bn_aggr(mv, stats)
                xc = small.tile([P, H], F32)
                nc.vector.tensor_scalar_sub(xc, h_ps, mv[:, 0:1])
                stdv = small.tile([P, 1], F32)
                nc.scalar.activation(
                    stdv, mv[:, 1:2], mybir.ActivationFunctionType.Sqrt, bias=eps_t
                )
                rstd = small.tile([P, 1], F32)
                nc.vector.reciprocal(rstd, stdv)
                xn = small.tile([P, H], F32)
                nc.vector.tensor_scalar_mul(xn, xc, rstd)
                sg = small.tile([P, H], F32)
                nc.scalar.activation(sg, xn, mybir.ActivationFunctionType.Sigmoid)
                nc.vector.tensor_mul(m_rep[:, 0, a * H:(a + 1) * H], xn, sg)

            # replicate ISUB times (doubling copies on DVE)
            rep = 1
            while rep < ISUB:
                cnt = min(rep, ISUB - rep)
                nc.vector.tensor_copy(
                    m_rep[:, rep:rep + cnt, :], m_rep[:, 0:cnt, :]
                )
                rep += cnt
            if debug_mrep and (debug_mrep & 1):
                nc.gpsimd.dma_start(out=dbg_d[:, :, :], in_=m_rep)

            # stage-1 PSUM pools stay open: releasing them would put a
            # (PE+DVE) release-wait on stage-2's first Matmult, which walrus
            # cannot encode.  4 stage-1 banks + 1 acc + 2 transpose banks = 7.
            # ------------- out[i,h] = sum_j mask[i,j,h] * m[j,h] -------------
            # acc2[h, i] += mt[:, ii, a*H:(a+1)*H].T @ ones  (partition-reduce
            # over j-quads via PE; the 4 a-slots accumulate in PSUM).  One
            # PSUM bank holds all ih columns.
            opsum_cm = tc.tile_pool(name="opsum", bufs=1, space="PSUM")
            opsum = opsum_cm.__enter__()
            tpsum_cm = tc.tile_pool(name="tpsum", bufs=2, space="PSUM")
            tpsum = tpsum_cm.__enter__()
            acc2 = opsum.tile([P, ih], F32)
            for rp in range(repeat):
              for it in range(nit):
                mt = loads.tile([P, ISUB, AQ * H], F32)
                src = mask_d[it * ISUB:(it + 1) * ISUB, :, :].rearrange(
                    "i (p a) h -> p i (a h)", a=AQ
                )
                nc.sync.dma_start(out=mt, in_=src)
                # ACT casts the f32 mask tile to bf16 (ACT is otherwise
                # idle); DVE then multiplies in bf16 at the 16-bit rate.
                mtb = prod.tile([P, ISUB, AQ * H], BF16)
                nc.scalar.copy(mtb, mt)
                if do_mul:
                    nc.vector.tensor_mul(mtb, mtb, m_rep)
                if (debug_mrep & 2) and it == 0:
                    dbg2 = nc.declare_dram_parameter(
                        "dbg2", [P, ISUB, AQ * H], F32, isOutput=True
                    )
                    nc.gpsimd.dma_start(out=dbg2[:, :, :], in_=mtb)
                # a outer, ii inner: consecutive matmuls hit different PSUM
                # columns, so the accumulate-into-same-column drain stall
                # (RAW on the PSUM address) only recurs every ISUB matmuls.
                for a in range(AQ if do_mm else 0):
                    for ii in range(ISUB):
                        i = it * ISUB + ii
                        # One accumulation group spans the whole bank: start
                        # zeroes the full 2KB zero region, so only the global
                        # first/last matmuls carry start/stop.
                        nc.tensor.matmul(
                            acc2[:, i:i + 1],
                            lhsT=mtb[:, ii, a * H:(a + 1) * H],
                            rhs=ones_col,
                            start=(it == 0 and a == 0 and ii == 0),
                            stop=(
                                it == nit - 1 and a == AQ - 1 and ii == ISUB - 1
                            ),
                        )
            # epilogue: acc2 is [h, i]; transpose 128-blocks back to [i, h]
            accT = outs.tile([P, ih], F32)
            if do_mm:
                nc.vector.tensor_copy(accT, acc2)
            else:
                nc.vector.memset(accT, 0.0)
                nc.vector.tensor_copy(acc2[:, 0:1], accT[:, 0:1])
            if debug_mrep & 4:
                dbg3 = nc.declare_dram_parameter(
                    "dbg3", [P, ih], F32, isOutput=True
                )
                nc.sync.dma_start(out=dbg3[:, :], in_=accT)
            for blk in range((ih + P - 1) // P):
                w = min(P, ih - blk * P)
                tp = tpsum.tile([w, P], F32)
                nc.tensor.transpose(tp, accT[:, blk * P:blk * P + w], ident)
                oT = outs.tile([w, P], F32)
                nc.scalar.copy(oT, tp)
                nc.sync.dma_start(out=out_d[blk * P:blk * P + w, :], in_=oT)
            tpsum_cm.__exit__(None, None, None)
            opsum_cm.__exit__(None, None, None)
            stage1_psum.__exit__(None, None, None)
    nc.finalize()
    return nc


_NC_CACHE = {}


def _get_nc():
    key = "main"
    if key not in _NC_CACHE:
        _NC_CACHE[key] = build_nc()
    return _NC_CACHE[key]


def kernel(s, ef_mask, W, b):
    s = np.ascontiguousarray(s, dtype=np.float32)
    ef_mask = np.ascontiguousarray(ef_mask, dtype=np.float32)
    W = np.ascontiguousarray(W, dtype=np.float32)
    b = np.ascontiguousarray(b, dtype=np.float32)

    nc = _get_nc()
    in_maps = []
    for c in range(N_CORES):
        bb = c // 2
        half = c % 2
        in_maps.append(
            {
                "s": s[bb],
                "w": W,
                "b": b,
                "mask": ef_mask[bb, half * IH:(half + 1) * IH],
            }
        )
    res = run_bass_kernel_spmd(nc, in_maps, list(range(N_CORES))).results
    out = np.empty((B, N, H), dtype=np.float32)
    for c in range(N_CORES):
        bb = c // 2
        half = c % 2
        out[bb, half * IH:(half + 1) * IH] = res[c]["out"]
    return out


# revision 17
# speedup vs baseline: 1.0163x; 1.0163x over previous
"""Trainium2 Bass kernel for DirCFConv-style GNN message passing.

Computes, for inputs s:(B,N,H) f32, ef_mask:(B,N,N,H) f32, W:(H,H), b:(H,):
    m   = SiLU(LayerNorm(s @ W.T + b))          # (B,N,H)
    out[b,i,h] = sum_j ef_mask[b,i,j,h] * m[b,j,h]

Sharding: 8 cores, core c handles batch b = c // 2 and query-node half
i in [ (c%2)*256, (c%2)*256+256 ).  Each core streams its 64 MiB mask
shard from HBM (memory-bound).

Mask layout: partition p holds the j-quad j in [4p, 4p+4), so each
per-partition DMA chunk is (a h) = 4*128 f32 = 2 KiB *contiguous* in
HBM (vs 512 B for the j-partition layout) - 4x fewer descriptors and
near-line-rate HBM reads.  m is reshuffled once into the matching
[p, (a h)] layout via a scratch-DRAM roundtrip, then replicated ISUB
times so one tensor_mul covers a whole mask tile.  The j-reduction is
4 PE matmuls per i (one per a-slot) accumulating into one PSUM column.
"""

import numpy as np

import concourse.bass as bass
import concourse.bacc as bacc
import concourse.tile as tile
from concourse import mybir
from concourse.bass_utils import run_bass_kernel_spmd
from concourse.masks import make_identity

B, N, H = 4, 512, 128
P = 128
NJT = N // P          # 4 j-tiles of 128
AQ = 4                # j's per partition (quad)
ISUB = 16             # i's per mask tile -> 4 MiB DMAs
IH = N // 2           # 256 i's per core
N_CORES = 8
LN_EPS = 1e-5
F32 = mybir.dt.float32
BF16 = mybir.dt.bfloat16


def build_nc(ih=IH, prod_dtype=F32, repeat=1, do_mul=True, do_mm=True,
             debug_mrep=False):
    nc = bacc.Bacc()
    s_d = nc.declare_dram_parameter("s", [N, H], F32, isOutput=False)
    w_d = nc.declare_dram_parameter("w", [H, H], F32, isOutput=False)
    b_d = nc.declare_dram_parameter("b", [H], F32, isOutput=False)
    mask_d = nc.declare_dram_parameter("mask", [ih, N, H], F32, isOutput=False)
    out_d = nc.declare_dram_parameter("out", [ih, H], F32, isOutput=True)
    dbg_d = None
    if debug_mrep and (debug_mrep & 1):
        dbg_d = nc.declare_dram_parameter(
            "dbg", [P, ISUB, AQ * H], F32, isOutput=True
        )

    nit = ih // ISUB
    with tile.TileContext(nc) as tc:
        with (
            tc.tile_pool(name="consts", bufs=1) as consts,
            tc.tile_pool(name="small", bufs=4) as small,
            tc.tile_pool(name="loads", bufs=3) as loads,
            tc.tile_pool(name="prod", bufs=3) as prod,
            tc.tile_pool(name="outs", bufs=3) as outs,
        ):
            stage1_psum = tc.tile_pool(name="spsum", bufs=1, space="PSUM")
            spsum = stage1_psum.__enter__()
            # ---------------- constants ----------------
            # All constants are produced on gpsimd BEFORE make_identity so the
            # single carrier wait (Pool sem) covers every one of them.
            ones_col = consts.tile([P, 1], BF16)
            nc.gpsimd.memset(ones_col, 1.0)
            ones_row = consts.tile([1, P], F32)
            nc.gpsimd.memset(ones_row, 1.0)
            # eps on DVE: its consumer (ACT Sqrt) already waits on DVE for mv,
            # and one DVE sem wait covers both (Activation also allows only 1).
            eps_t = consts.tile([P, 1], F32)
            nc.vector.memset(eps_t, LN_EPS)
            ident = consts.tile([P, P], F32)
            make_identity(nc, ident)

            # Small parameter loads ride the ACT HWDGE ring so the SP ring
            # carries only the big mask streams (first mask DMA starts at t=0).
            w_sb = consts.tile([H, H], F32)
            nc.scalar.dma_start(out=w_sb, in_=w_d[:, :])
            bias_sb = consts.tile([1, H], F32)
            b_ap = b_d[:]
            bias_src = bass.AP(
                tensor=b_ap.tensor, offset=b_ap.offset, ap=[[0, 1]] + list(b_ap.ap)
            )
            nc.scalar.dma_start(out=bias_sb, in_=bias_src)

            # Wait-carrier: walrus allows only ONE sync wait per Matmult, so
            # absorb the gpsimd(identity) dependency into a throwaway PE op;
            # later matmuls then only carry their own single DMA/engine wait.
            carrier_ps = spsum.tile([P, P], F32)
            nc.tensor.transpose(carrier_ps, ident, ident)

            # W^T via PE-transpose: (o,h) -> (h,o)
            wT_ps = spsum.tile([H, H], F32)
            nc.tensor.transpose(wT_ps, w_sb, ident)
            wT_sb = consts.tile([H, H], F32)
            nc.scalar.copy(wT_sb, wT_ps)

            # ------------- m = SiLU(LN(s @ W.T + b)) -------------
            # All four s^T blocks share one PSUM bank (one zero-region group);
            # likewise the four h = s@W.T+b blocks.  No PSUM slot rotation ->
            # no extra release waits on any Matmult.
            sT_all = spsum.tile([P, NJT * P], F32)
            h_all = spsum.tile([P, NJT * H], F32)
            s_sbs = []
            for jt in range(NJT):
                s_sb = small.tile([P, H], F32, tag=f"s_sb{jt}")
                nc.scalar.dma_start(out=s_sb, in_=s_d[jt * P:(jt + 1) * P, :])
                s_sbs.append(s_sb)
                nc.tensor.matmul(
                    sT_all[:, jt * P:(jt + 1) * P],
                    lhsT=s_sb,
                    rhs=ident,
                    is_transpose=True,
                    start=(jt == 0),
                    stop=(jt == NJT - 1),
                )
            sT_sb = consts.tile([P, NJT * P], F32)
            nc.scalar.copy(sT_sb, sT_all)
            # h_quad[p, a*H+h] = h[4p+a, h]: the a-th matmul uses the
            # free-dim-strided stationary sT_sb[:, a::4] (columns j=4p+a),
            # so m lands directly in the j-quad layout the mask tiles use.
            for a in range(AQ):
                nc.tensor.matmul(
                    h_all[:, a * H:(a + 1) * H],
                    lhsT=sT_sb[:, a::AQ],
                    rhs=wT_sb,
                    start=(a == 0),
                    stop=False,
                )
                nc.tensor.matmul(
                    h_all[:, a * H:(a + 1) * H],
                    lhsT=ones_row,
                    rhs=bias_sb,
                    start=False,
                    stop=(a == AQ - 1),
                )

            # LN + SiLU per a-slot, writing m_rep[p, 0, (a h)] = m[4p+a, h]
            # (bf16: the product path runs in bf16 so PE stationary loads are
            # 1 cyc/row instead of fp32r's 4, and DVE gets the 16-bit rate)
            m_rep = consts.tile([P, ISUB, AQ * H], BF16)
            for a in range(AQ):
                h_ps = h_all[:, a * H:(a + 1) * H]
                stats = small.tile([P, 6], F32)
                nc.vector.bn_stats(stats, h_ps)
                mv = small.tile([P, 2], F32)
                nc.vector.bn_aggr(mv, stats)
                xc = small.tile([P, H], F32)
                nc.vector.tensor_scalar_sub(xc, h_ps, mv[:, 0:1])
                stdv = small.tile([P, 1], F32)
                nc.scalar.activation(
                    stdv, mv[:, 1:2], mybir.ActivationFunctionType.Sqrt, bias=eps_t
                )
                rstd = small.tile([P, 1], F32)
                nc.vector.reciprocal(rstd, stdv)
                xn = small.tile([P, H], F32)
                nc.vector.tensor_scalar_mul(xn, xc, rstd)
                sg = small.tile([P, H], F32)
                nc.scalar.activation(sg, xn, mybir.ActivationFunctionType.Sigmoid)
                nc.vector.tensor_mul(m_rep[:, 0, a * H:(a + 1) * H], xn, sg)

            # replicate ISUB times (doubling copies on DVE)
            rep = 1
            while rep < ISUB:
                cnt = min(rep, ISUB - rep)
                nc.vector.tensor_copy(
                    m_rep[:, rep:rep + cnt, :], m_rep[:, 0:cnt, :]
                )
                rep += cnt
            if debug_mrep and (debug_mrep & 1):
                nc.gpsimd.dma_start(out=dbg_d[:, :, :], in_=m_rep)

            # stage-1 PSUM pools stay open: releasing them would put a
            # (PE+DVE) release-wait on stage-2's first Matmult, which walrus
            # cannot encode.  4 stage-1 banks + 1 acc + 2 transpose banks = 7.
            # ------------- out[i,h] = sum_j mask[i,j,h] * m[j,h] -------------
            # acc2[h, i] += mt[:, ii, a*H:(a+1)*H].T @ ones  (partition-reduce
            # over j-quads via PE; the 4 a-slots accumulate in PSUM).  One
            # PSUM bank holds all ih columns.
            opsum_cm = tc.tile_pool(name="opsum", bufs=1, space="PSUM")
            opsum = opsum_cm.__enter__()
            tpsum_cm = tc.tile_pool(name="tpsum", bufs=2, space="PSUM")
            tpsum = tpsum_cm.__enter__()
            acc2 = opsum.tile([P, ih], F32)
            for rp in range(repeat):
              for it in range(nit):
                mt = loads.tile([P, ISUB, AQ * H], F32)
                src = mask_d[it * ISUB:(it + 1) * ISUB, :, :].rearrange(
                    "i (p a) h -> p i (a h)", a=AQ
                )
                nc.sync.dma_start(out=mt, in_=src)
                # ACT casts the f32 mask tile to bf16 (ACT is otherwise
                # idle); DVE then multiplies in bf16 at the 16-bit rate.
                mtb = prod.tile([P, ISUB, AQ * H], BF16)
                nc.scalar.copy(mtb, mt)
                if do_mul:
                    nc.vector.tensor_mul(mtb, mtb, m_rep)
                if (debug_mrep & 2) and it == 0:
                    dbg2 = nc.declare_dram_parameter(
                        "dbg2", [P, ISUB, AQ * H], F32, isOutput=True
                    )
                    nc.gpsimd.dma_start(out=dbg2[:, :, :], in_=mtb)
                # a outer, ii inner: consecutive matmuls hit different PSUM
                # columns, so the accumulate-into-same-column drain stall
                # (RAW on the PSUM address) only recurs every ISUB matmuls.
                for a in range(AQ if do_mm else 0):
                    for ii in range(ISUB):
                        i = it * ISUB + ii
                        # One accumulation group spans the whole bank: start
                        # zeroes the full 2KB zero region, so only the global
                        # first/last matmuls carry start/stop.
                        nc.tensor.matmul(
                            acc2[:, i:i + 1],
                            lhsT=mtb[:, ii, a * H:(a + 1) * H],
                            rhs=ones_col,
                            start=(it == 0 and a == 0 and ii == 0),
                            stop=(
                                it == nit - 1 and a == AQ - 1 and ii == ISUB - 1
                            ),
                        )
            # epilogue: acc2 is [h, i]; transpose 128-blocks back to [i, h]
            accT = outs.tile([P, ih], F32)
            if do_mm:
                nc.vector.tensor_copy(accT, acc2)
            else:
                nc.vector.memset(accT, 0.0)
                nc.vector.tensor_copy(acc2[:, 0:1], accT[:, 0:1])
            if debug_mrep & 4:
                dbg3 = nc.declare_dram_parameter(
                    "dbg3", [P, ih], F32, isOutput=True
                )
                nc.sync.dma_start(out=dbg3[:, :], in_=accT)
            for blk in range((ih + P - 1) // P):
                w = min(P, ih - blk * P)
                tp = tpsum.tile([w, P], F32)
                nc.tensor.transpose(tp, accT[:, blk * P:blk * P + w], ident)
                oT = outs.tile([w, P], F32)
                nc.scalar.copy(oT, tp)
                nc.sync.dma_start(out=out_d[blk * P:blk * P + w, :], in_=oT)
            tpsum_cm.__exit__(None, None, None)
            opsum_cm.__exit__(None, None, None)
            stage1_psum.__exit__(None, None, None)
    nc.finalize()
    return nc


_NC_CACHE = {}


def _get_nc():
    key = "main"
    if key not in _NC_CACHE:
        _NC_CACHE[key] = build_nc()
    return _NC_CACHE[key]


def kernel(s, ef_mask, W, b):
    s = np.ascontiguousarray(s, dtype=np.float32)
    ef_mask = np.ascontiguousarray(ef_mask, dtype=np.float32)
    W = np.ascontiguousarray(W, dtype=np.float32)
    b = np.ascontiguousarray(b, dtype=np.float32)

    nc = _get_nc()
    in_maps = []
    for c in range(N_CORES):
        bb = c // 2
        half = c % 2
        in_maps.append(
            {
                "s": s[bb],
                "w": W,
                "b": b,
                "mask": ef_mask[bb, half * IH:(half + 1) * IH],
            }
        )
    res = run_bass_kernel_spmd(nc, in_maps, list(range(N_CORES))).results
    out = np.empty((B, N, H), dtype=np.float32)
    for c in range(N_CORES):
        bb = c // 2
        half = c % 2
        out[bb, half * IH:(half + 1) * IH] = res[c]["out"]
    return out
